# revision 11
# baseline (speedup 1.0000x reference)
"""Multi-head attention TRN2 kernel, head-parallel across 8 NeuronCores.

Per core c (= head h=c), all matmuls in bf16 (PE streams 1 col/cycle for
bf16 and f32r alike, but bf16 halves DMA bytes, enables FWL fast weight
loads, and doubles DVE throughput), keys-on-partitions score layout,
with both outer projections algebraically fused into the K / V
projections:

  scores = q Wq (k Wk)^T = q G k^T          G = Wq Wk^T   (host)
  out    = attn (v Wv) Wo = attn (v U)      U = Wv Wo_h   (host)

so the device only computes, per core:

  K2T[d,t] = A k^T   with A = G^T = Wk Wq^T  (lhsT = A nat, rhs = kT)
  V2[t,o]  = v U                             (lhsT = vT,  rhs = U)
  scoresT[t,s] = K2 q^T                      (lhsT = K2T, rhs = qT chunk)
  E = exp(scoresT*scale + Madd + wbias[t])   (DVE mask-add, ACT exp)
  rowsum tree (DVE, f32) -> rs partials to host
  outT[o,s] = V2^T E                         (lhsT = V2, rhs = E)

Schedule (tuned from traces; PE is the bottleneck at ~216 ns/MM for
N=512, so everything else must hide):
  - 14 warmup matmuls on a memset tile keep the PE busy from the end of
    the framework preamble (~7.5us) until the first input chunk lands.
  - Mask DMAs ride the sync HWDGE ring FIFO behind the stage-A input
    loads: SDMA engines round-robin between queues at packet
    granularity, so a concurrent SWDGE mask stream would crowd the
    input rings out of HBM bandwidth exactly when the PE is starved.
  - All stage-A input DMAs are hoisted to program start and interleaved
    across BOTH hardware DGE rings (sync and scalar) by parity, so
    their issue instructions are not stuck behind dependent compute in
    an engine FIFO and the two rings drain in parallel.
  - Stage A computes K2T/V2 for both batches up front; stage B then
    iterates mask-chunk-major so the (batch-independent) mask is
    loaded once, not once per batch.
  - Stage B AV runs os-major with per-os PSUM drain, so output copies
    and DMAs trickle out during accumulation instead of bursting at
    the end; the last unit's outputs go on the fast HWDGE rings.

Host: transposes q/k/v, casts bf16 (RNE), builds additive bf16 mask
(0 / -1e9) in [t,s] orientation, folds all biases exactly (bk drops
under softmax; bq -> per-key exp bias; bv,bo -> final add), sums
per-head partial outputs, divides by the gathered rowsum partials, and
transposes the [dout, s] device layout back to [b, s, dout].
"""
import sys
import numpy as np

sys.path.insert(0, "/opt/trn_rl_repo")

H, D, B, S = 8, 512, 2, 2048
P = 128
NE = D // P            # 4 feature tiles
NT = S // P            # 16 key tiles per batch
CH = 512               # query/key chunk width
NCH = S // CH          # 4 chunks per batch
SCALE = 1.0 / np.sqrt(np.float32(D))

_CACHE = {}


def _build():
    from contextlib import ExitStack
    from concourse import bass, bacc, tile

    mybir = bass.mybir
    dt = mybir.dt
    AF = mybir.ActivationFunctionType
    ALU = mybir.AluOpType

    nc = bacc.Bacc("TRN2", target_bir_lowering=False, debug=False)

    qT_d = nc.dram_tensor("qT", [D, B * S], dt.bfloat16, kind="ExternalInput")
    kT_d = nc.dram_tensor("kT", [D, B * S], dt.bfloat16, kind="ExternalInput")
    vT_d = nc.dram_tensor("vT", [D, B * S], dt.bfloat16, kind="ExternalInput")
    mT_d = nc.dram_tensor("mT", [S, S], dt.bfloat16, kind="ExternalInput")
    ka_d = nc.dram_tensor("ka", [D, D], dt.bfloat16, kind="ExternalInput")  # Wk Wq^T
    vu_d = nc.dram_tensor("vu", [D, D], dt.bfloat16, kind="ExternalInput")  # Wv Wo_h
    wb_d = nc.dram_tensor("wb", [P, B * NT], dt.float32, kind="ExternalInput")
    out_d = nc.dram_tensor("out", [D, B * S], dt.bfloat16, kind="ExternalOutput")
    rs_d = nc.dram_tensor("rs", [P, B * S], dt.float32, kind="ExternalOutput")

    def dtiles(ap_2d):
        return ap_2d.rearrange("(a p) c -> p a c", p=P)

    with tile.TileContext(nc) as tc:
        with ExitStack() as ctx:
            wpool = ctx.enter_context(tc.tile_pool(name="w", bufs=1))
            kvpool = ctx.enter_context(tc.tile_pool(name="kv", bufs=1))
            kpool = ctx.enter_context(tc.tile_pool(name="kin", bufs=4))
            vpool = ctx.enter_context(tc.tile_pool(name="vin", bufs=4))
            qpool = ctx.enter_context(tc.tile_pool(name="qin", bufs=2))
            epool = ctx.enter_context(tc.tile_pool(name="e", bufs=2))
            mpool = ctx.enter_context(tc.tile_pool(name="m", bufs=2))
            tpool = ctx.enter_context(tc.tile_pool(name="tmp", bufs=3))
            rpool = ctx.enter_context(tc.tile_pool(name="r", bufs=1))
            apool = ctx.enter_context(tc.tile_pool(name="a", bufs=2))
            opool = ctx.enter_context(tc.tile_pool(name="o", bufs=4))
            psA = ctx.enter_context(tc.tile_pool(name="psA", bufs=4, space="PSUM"))
            psO = ctx.enter_context(tc.tile_pool(name="psO", bufs=4, space="PSUM"))

            # ---- PE warmup: keep the array busy from preamble end until
            # the first input chunk lands (also starts HAM un-throttling) ----
            warm = wpool.tile([P, CH], dt.bfloat16)
            nc.vector.memset(warm[:], 0.0)
            for i in range(14):
                pw = psA.tile([P, CH], dt.float32, tag="ps")
                nc.tensor.matmul(pw[:], warm[:, 0:P], warm[:], start=True, stop=True)

            ka = wpool.tile([P, NE, D], dt.bfloat16)
            vu = wpool.tile([P, NE, D], dt.bfloat16)
            wb = wpool.tile([P, B * NT], dt.float32)
            nc.scalar.dma_start(ka[:], dtiles(ka_d.ap()))
            nc.scalar.dma_start(vu[:], dtiles(vu_d.ap()))
            nc.sync.dma_start(wb[:], wb_d[:])

            K2T = kvpool.tile([P, B, NE, S], dt.bfloat16, tag="K2T")
            V2 = kvpool.tile([P, B, NT, D], dt.bfloat16, tag="V2")

            qTt = dtiles(qT_d.ap())
            kTt = dtiles(kT_d.ap())
            vTt = dtiles(vT_d.ap())
            mTt = mT_d.ap().rearrange("(a p) c -> p a c", p=P)  # [128, NT, S]

            # ---- hoisted input prefetch: batch-0 k/v chunks, batch-1 kin c0,
            # interleaved across both HWDGE rings by parity ----
            kin_t = {}
            vin_t = {}
            for c4 in range(NCH):
                kin = kpool.tile([P, NE, CH], dt.bfloat16, tag="kin")
                nc.sync.dma_start(kin[:], kTt[:, :, c4 * CH:(c4 + 1) * CH])
                kin_t[(0, c4)] = kin
            for c4 in range(NCH):
                vin = vpool.tile([P, NE, CH], dt.bfloat16, tag="vin")
                nc.scalar.dma_start(vin[:], vTt[:, :, c4 * CH:(c4 + 1) * CH])
                vin_t[(0, c4)] = vin
            kin = kpool.tile([P, NE, CH], dt.bfloat16, tag="kin")
            nc.sync.dma_start(kin[:], kTt[:, :, S:S + CH])
            kin_t[(1, 0)] = kin

            # ---- stage A: K2^T and V2 for both batches ----
            for b in range(B):
                for c4 in range(NCH):
                    col0 = b * S + c4 * CH
                    kin = kin_t.get((b, c4))
                    if kin is None:
                        kin = kpool.tile([P, NE, CH], dt.bfloat16, tag="kin")
                        nc.sync.dma_start(kin[:], kTt[:, :, col0:col0 + CH])
                    for et in range(NE):
                        ps = psA.tile([P, CH], dt.float32, tag="ps")
                        for kd in range(NE):
                            nc.tensor.matmul(
                                ps[:], ka[:, kd, et * P:(et + 1) * P], kin[:, kd, :],
                                start=(kd == 0), stop=(kd == NE - 1))
                        nc.scalar.copy(K2T[:, b, et, c4 * CH:(c4 + 1) * CH], ps[:])
                for c4 in range(NCH):
                    col0 = b * S + c4 * CH
                    vin = vin_t.get((b, c4))
                    if vin is None:
                        vin = vpool.tile([P, NE, CH], dt.bfloat16, tag="vin")
                        nc.scalar.dma_start(vin[:], vTt[:, :, col0:col0 + CH])
                    for ts in range(CH // P):
                        ps = psA.tile([P, D], dt.float32, tag="ps")
                        for kd in range(NE):
                            nc.tensor.matmul(
                                ps[:], vin[:, kd, ts * P:(ts + 1) * P], vu[:, kd, :],
                                start=(kd == 0), stop=(kd == NE - 1))
                        nc.scalar.copy(V2[:, b, c4 * (CH // P) + ts, :], ps[:])

            # ---- stage B: mask-chunk-major attention (mask loaded once) ----
            units = [(c, b) for c in range(NCH) for b in range(B)]
            qin_t = {}
            mt_t = {}

            def prefetch_qin(i):
                if i < len(units):
                    c, b = units[i]
                    qin = qpool.tile([P, NE, CH], dt.bfloat16, tag="qin")
                    nc.scalar.dma_start(
                        qin[:], qTt[:, :, b * S + c * CH:b * S + (c + 1) * CH])
                    qin_t[(c, b)] = qin

            def prefetch_mask(c):
                # on the sync HWDGE ring: queues FIFO behind the stage-A kin
                # loads, so the 2 MB mask transfers never compete with the
                # critical early input DMAs for HBM bandwidth
                if c < NCH and c not in mt_t:
                    mt = mpool.tile([P, NT, CH], dt.bfloat16)
                    nc.sync.dma_start(mt[:], mTt[:, :, c * CH:(c + 1) * CH])
                    mt_t[c] = mt

            prefetch_mask(0)
            prefetch_mask(1)
            prefetch_qin(0)
            prefetch_qin(1)

            pending = []

            def drain_one(alt):
                # deferred PSUM->SBUF drain of the previous unit's AV tile,
                # interleaved between this unit's exps so the ACT FIFO never
                # makes next-unit exps wait behind AV-gated copies
                if pending:
                    pso, pcol, pos = pending.pop(0)
                    ot = opool.tile([P, CH], dt.bfloat16)
                    nc.scalar.copy(ot[:], pso[:])
                    r0 = pos * P
                    eng = nc.sync if (pos % 2 == 0) else nc.gpsimd
                    eng.dma_start(out_d[r0:r0 + P, pcol:pcol + CH], ot[:])

            for i, (c, b) in enumerate(units):
                prefetch_qin(i + 2)
                if b == 0:
                    prefetch_mask(c + 1)
                mt = mt_t[c]
                col0 = b * S + c * CH
                qin = qin_t[(c, b)]
                last = (i >= len(units) - 2)

                E = epool.tile([P, NT, CH], dt.bfloat16, tag="E")
                for tt in range(NT):
                    ps = psA.tile([P, CH], dt.float32, tag="ps")
                    for et in range(NE):
                        nc.tensor.matmul(
                            ps[:], K2T[:, b, et, tt * P:(tt + 1) * P], qin[:, et, :],
                            start=(et == 0), stop=(et == NE - 1))
                    tmp = tpool.tile([P, CH], dt.float32)
                    nc.vector.scalar_tensor_tensor(
                        tmp[:], ps[:], float(SCALE), mt[:, tt, :],
                        op0=ALU.mult, op1=ALU.add)
                    nc.scalar.activation(
                        E[:, tt, :], tmp[:], AF.Exp,
                        bias=wb[:, b * NT + tt: b * NT + tt + 1], scale=1.0)
                    if tt in (1, 3, 5, 7):
                        drain_one(tt)

                # rowsum tree on gpsimd: keeps the DVE FIFO free for the
                # scalar_tensor_tensor drains that gate psA buffer reuse
                red = rpool.tile([P, NE, CH], dt.float32, tag="red")
                nc.gpsimd.tensor_add(red[:], E[:, 0:4, :], E[:, 4:8, :])
                nc.gpsimd.tensor_add(red[:], red[:], E[:, 8:12, :])
                nc.gpsimd.tensor_add(red[:], red[:], E[:, 12:16, :])
                nc.gpsimd.tensor_add(red[:, 0:2, :], red[:, 0:2, :], red[:, 2:4, :])
                accr = apool.tile([P, CH], dt.float32, tag="accr")
                nc.gpsimd.tensor_add(accr[:], red[:, 0, :], red[:, 1, :])
                rse = nc.sync if last else nc.gpsimd
                rse.dma_start(rs_d[:, col0:col0 + CH], accr[:])

                for os_ in range(NE):
                    pso = psO.tile([P, CH], dt.float32, tag="pso")
                    for tt in range(NT):
                        nc.tensor.matmul(
                            pso[:], V2[:, b, tt, os_ * P:(os_ + 1) * P],
                            E[:, tt, :],
                            start=(tt == 0), stop=(tt == NT - 1))
                    r0 = os_ * P
                    if i == len(units) - 1:
                        # final unit drains immediately; last tile ships in
                        # halves on both HWDGE rings for the shortest tail
                        ot = opool.tile([P, CH], dt.bfloat16)
                        if os_ == NE - 1:
                            for hh in range(2):
                                sl = slice(hh * (CH // 2), (hh + 1) * (CH // 2))
                                nc.scalar.copy(ot[:, sl], pso[:, sl])
                                eng = nc.sync if hh == 0 else nc.scalar
                                eng.dma_start(
                                    out_d[r0:r0 + P, col0 + hh * (CH // 2):
                                          col0 + (hh + 1) * (CH // 2)], ot[:, sl])
                        else:
                            nc.scalar.copy(ot[:], pso[:])
                            eng = nc.sync if (os_ % 2 == 0) else nc.scalar
                            eng.dma_start(out_d[r0:r0 + P, col0:col0 + CH], ot[:])
                    else:
                        pending.append((pso, col0, os_))

    nc.compile()
    return nc


def kernel(q, k, v, mask, Wq, bq, Wk, bk, Wv, bv, Wo, bo):
    from concourse.bass_utils import run_bass_kernel_spmd
    import ml_dtypes

    bf16 = ml_dtypes.bfloat16

    q = np.asarray(q, np.float32)
    k = np.asarray(k, np.float32)
    v = np.asarray(v, np.float32)
    mask = np.asarray(mask)
    Wq = np.asarray(Wq, np.float32)
    Wk = np.asarray(Wk, np.float32)
    Wv = np.asarray(Wv, np.float32)
    Wo = np.asarray(Wo, np.float32)
    bq = np.asarray(bq, np.float32)
    bk = np.asarray(bk, np.float32)
    bv = np.asarray(bv, np.float32)
    bo = np.asarray(bo, np.float32)

    qT = np.ascontiguousarray(q.transpose(2, 0, 1).reshape(D, B * S)).astype(bf16)
    kT = np.ascontiguousarray(k.transpose(2, 0, 1).reshape(D, B * S)).astype(bf16)
    vT = np.ascontiguousarray(v.transpose(2, 0, 1).reshape(D, B * S)).astype(bf16)
    mT = np.where(mask.T == 1, np.float32(-1e9), np.float32(0.0)).astype(bf16)
    mT = np.ascontiguousarray(mT)

    kf = k.reshape(B * S, D)
    in_maps = []
    for h in range(H):
        Wq64 = Wq[h].astype(np.float64)
        Wk64 = Wk[h].astype(np.float64)
        Wv64 = Wv[h].astype(np.float64)
        Wo64 = Wo[h * D:(h + 1) * D, :].astype(np.float64)
        A = (Wk64 @ Wq64.T).astype(np.float32)       # lhsT for K2^T proj
        U = (Wv64 @ Wo64).astype(np.float32)         # rhs for V2 proj
        wvec = (kf @ (Wk[h] @ bq[h])) * SCALE        # per-key exp bias
        wb = np.ascontiguousarray(wvec.reshape(B * NT, P).T.astype(np.float32))
        in_maps.append({
            "qT": qT, "kT": kT, "vT": vT, "mT": mT,
            "ka": A.astype(bf16), "vu": U.astype(bf16), "wb": wb,
        })

    if "nc" not in _CACHE:
        _CACHE["nc"] = _build()
    nc = _CACHE["nc"]
    _CACHE["in_maps"] = in_maps

    res = run_bass_kernel_spmd(nc, in_maps, core_ids=list(range(H)))
    total = np.zeros((D, B * S), np.float64)
    for h in range(H):
        r = res.results[h]["rs"].sum(axis=0, dtype=np.float64)   # [B*S]
        total += res.results[h]["out"].astype(np.float64) / r[None, :]

    cvec = bo.astype(np.float64).copy()
    for h in range(H):
        cvec += bv[h].astype(np.float64) @ Wo[h * D:(h + 1) * D, :].astype(np.float64)
    total += cvec[:, None]
    return total.T.astype(np.float32).reshape(B, S, D)


# revision 14
# speedup vs baseline: 1.0010x; 1.0010x over previous
"""Multi-head attention TRN2 kernel, head-parallel across 8 NeuronCores.

Per core c (= head h=c), all matmuls in bf16 (PE streams 1 col/cycle for
bf16 and f32r alike, but bf16 halves DMA bytes, enables FWL fast weight
loads, and doubles DVE throughput), keys-on-partitions score layout,
with both outer projections algebraically fused into the K / V
projections:

  scores = q Wq (k Wk)^T = q G k^T          G = Wq Wk^T   (host)
  out    = attn (v Wv) Wo = attn (v U)      U = Wv Wo_h   (host)

so the device only computes, per core:

  K2T[d,t] = A k^T   with A = G^T = Wk Wq^T  (lhsT = A nat, rhs = kT)
  V2[t,o]  = v U                             (lhsT = vT,  rhs = U)
  scoresT[t,s] = K2 q^T                      (lhsT = K2T, rhs = qT chunk)
  E = exp(scoresT*scale + Madd + wbias[t])   (DVE mask-add, ACT exp)
  rowsum tree (DVE, f32) -> rs partials to host
  outT[o,s] = V2^T E                         (lhsT = V2, rhs = E)

Schedule (tuned from traces; PE is the bottleneck at ~216 ns/MM for
N=512, so everything else must hide):
  - 14 warmup matmuls on a memset tile keep the PE busy from the end of
    the framework preamble (~7.5us) until the first input chunk lands.
  - Mask DMAs ride the sync HWDGE ring FIFO behind the stage-A input
    loads: SDMA engines round-robin between queues at packet
    granularity, so a concurrent SWDGE mask stream would crowd the
    input rings out of HBM bandwidth exactly when the PE is starved.
  - All stage-A input DMAs are hoisted to program start and interleaved
    across BOTH hardware DGE rings (sync and scalar) by parity, so
    their issue instructions are not stuck behind dependent compute in
    an engine FIFO and the two rings drain in parallel.
  - Stage A computes K2T/V2 for both batches up front; stage B then
    iterates mask-chunk-major so the (batch-independent) mask is
    loaded once, not once per batch.
  - Stage B AV runs os-major with per-os PSUM drain, so output copies
    and DMAs trickle out during accumulation instead of bursting at
    the end; the last unit's outputs go on the fast HWDGE rings.

Host: transposes q/k/v, casts bf16 (RNE), builds additive bf16 mask
(0 / -1e9) in [t,s] orientation, folds all biases exactly (bk drops
under softmax; bq -> per-key exp bias; bv,bo -> final add), sums
per-head partial outputs, divides by the gathered rowsum partials, and
transposes the [dout, s] device layout back to [b, s, dout].
"""
import sys
import numpy as np

sys.path.insert(0, "/opt/trn_rl_repo")

H, D, B, S = 8, 512, 2, 2048
P = 128
NE = D // P            # 4 feature tiles
NT = S // P            # 16 key tiles per batch
CH = 512               # query/key chunk width
NCH = S // CH          # 4 chunks per batch
SCALE = 1.0 / np.sqrt(np.float32(D))

_CACHE = {}


def _build():
    from contextlib import ExitStack
    from concourse import bass, bacc, tile

    mybir = bass.mybir
    dt = mybir.dt
    AF = mybir.ActivationFunctionType
    ALU = mybir.AluOpType

    nc = bacc.Bacc("TRN2", target_bir_lowering=False, debug=False)

    qT_d = nc.dram_tensor("qT", [D, B * S], dt.bfloat16, kind="ExternalInput")
    kT_d = nc.dram_tensor("kT", [D, B * S], dt.bfloat16, kind="ExternalInput")
    vT_d = nc.dram_tensor("vT", [D, B * S], dt.bfloat16, kind="ExternalInput")
    mT_d = nc.dram_tensor("mT", [S, S], dt.bfloat16, kind="ExternalInput")
    ka_d = nc.dram_tensor("ka", [D, D], dt.bfloat16, kind="ExternalInput")  # Wk Wq^T
    vu_d = nc.dram_tensor("vu", [D, D], dt.bfloat16, kind="ExternalInput")  # Wv Wo_h
    wb_d = nc.dram_tensor("wb", [P, B * NT], dt.float32, kind="ExternalInput")
    out_d = nc.dram_tensor("out", [D, B * S], dt.bfloat16, kind="ExternalOutput")
    rs_d = nc.dram_tensor("rs", [P, B * S], dt.float32, kind="ExternalOutput")

    def dtiles(ap_2d):
        return ap_2d.rearrange("(a p) c -> p a c", p=P)

    with tile.TileContext(nc) as tc:
        with ExitStack() as ctx:
            wpool = ctx.enter_context(tc.tile_pool(name="w", bufs=1))
            kvpool = ctx.enter_context(tc.tile_pool(name="kv", bufs=1))
            kpool = ctx.enter_context(tc.tile_pool(name="kin", bufs=4))
            vpool = ctx.enter_context(tc.tile_pool(name="vin", bufs=4))
            qpool = ctx.enter_context(tc.tile_pool(name="qin", bufs=2))
            epool = ctx.enter_context(tc.tile_pool(name="e", bufs=2))
            mpool = ctx.enter_context(tc.tile_pool(name="m", bufs=2))
            tpool = ctx.enter_context(tc.tile_pool(name="tmp", bufs=3))
            rpool = ctx.enter_context(tc.tile_pool(name="r", bufs=1))
            apool = ctx.enter_context(tc.tile_pool(name="a", bufs=2))
            opool = ctx.enter_context(tc.tile_pool(name="o", bufs=4))
            psA = ctx.enter_context(tc.tile_pool(name="psA", bufs=4, space="PSUM"))
            psO = ctx.enter_context(tc.tile_pool(name="psO", bufs=4, space="PSUM"))

            # ---- PE warmup: keep the array busy from preamble end until
            # the first input chunk lands (also starts HAM un-throttling) ----
            warm = wpool.tile([P, CH], dt.bfloat16)
            nc.vector.memset(warm[:], 0.0)
            for i in range(15):
                pw = psA.tile([P, CH], dt.float32, tag="ps")
                nc.tensor.matmul(pw[:], warm[:, 0:P], warm[:], start=True, stop=True)

            ka = wpool.tile([P, NE, D], dt.bfloat16)
            vu = wpool.tile([P, NE, D], dt.bfloat16)
            wb = wpool.tile([P, B * NT], dt.float32)

            K2T = kvpool.tile([P, B, NE, S], dt.bfloat16, tag="K2T")
            V2 = kvpool.tile([P, B, NT, D], dt.bfloat16, tag="V2")

            qTt = dtiles(qT_d.ap())
            kTt = dtiles(kT_d.ap())
            vTt = dtiles(vT_d.ap())
            mTt = mT_d.ap().rearrange("(a p) c -> p a c", p=P)  # [128, NT, S]

            # ---- hoisted input prefetch.  Ring heads ordered for the
            # startup critical path: the first K2 matmul needs ka + kin c0,
            # so kin c0 is split in halves across BOTH HWDGE rings and ka
            # leads the scalar ring; everything else queues behind. ----
            kin_t = {}
            vin_t = {}
            nc.scalar.dma_start(ka[:], dtiles(ka_d.ap()))
            nc.scalar.dma_start(vu[:], dtiles(vu_d.ap()))
            for c4 in range(NCH):
                kin = kpool.tile([P, NE, CH], dt.bfloat16, tag="kin")
                nc.sync.dma_start(kin[:], kTt[:, :, c4 * CH:(c4 + 1) * CH])
                kin_t[(0, c4)] = kin
            for c4 in range(NCH):
                vin = vpool.tile([P, NE, CH], dt.bfloat16, tag="vin")
                nc.scalar.dma_start(vin[:], vTt[:, :, c4 * CH:(c4 + 1) * CH])
                vin_t[(0, c4)] = vin
            kin = kpool.tile([P, NE, CH], dt.bfloat16, tag="kin")
            nc.sync.dma_start(kin[:], kTt[:, :, S:S + CH])
            kin_t[(1, 0)] = kin
            nc.sync.dma_start(wb[:], wb_d[:])

            # ---- stage A: K2^T and V2 for both batches ----
            for b in range(B):
                for c4 in range(NCH):
                    col0 = b * S + c4 * CH
                    kin = kin_t.get((b, c4))
                    if kin is None:
                        kin = kpool.tile([P, NE, CH], dt.bfloat16, tag="kin")
                        nc.sync.dma_start(kin[:], kTt[:, :, col0:col0 + CH])
                    for et in range(NE):
                        ps = psA.tile([P, CH], dt.float32, tag="ps")
                        for kd in range(NE):
                            nc.tensor.matmul(
                                ps[:], ka[:, kd, et * P:(et + 1) * P], kin[:, kd, :],
                                start=(kd == 0), stop=(kd == NE - 1))
                        nc.scalar.copy(K2T[:, b, et, c4 * CH:(c4 + 1) * CH], ps[:])
                for c4 in range(NCH):
                    col0 = b * S + c4 * CH
                    vin = vin_t.get((b, c4))
                    if vin is None:
                        vin = vpool.tile([P, NE, CH], dt.bfloat16, tag="vin")
                        nc.scalar.dma_start(vin[:], vTt[:, :, col0:col0 + CH])
                    for ts in range(CH // P):
                        ps = psA.tile([P, D], dt.float32, tag="ps")
                        for kd in range(NE):
                            nc.tensor.matmul(
                                ps[:], vin[:, kd, ts * P:(ts + 1) * P], vu[:, kd, :],
                                start=(kd == 0), stop=(kd == NE - 1))
                        nc.scalar.copy(V2[:, b, c4 * (CH // P) + ts, :], ps[:])

            # ---- stage B: mask-chunk-major attention (mask loaded once) ----
            units = [(c, b) for c in range(NCH) for b in range(B)]
            qin_t = {}
            mt_t = {}

            def prefetch_qin(i):
                if i < len(units):
                    c, b = units[i]
                    qin = qpool.tile([P, NE, CH], dt.bfloat16, tag="qin")
                    nc.scalar.dma_start(
                        qin[:], qTt[:, :, b * S + c * CH:b * S + (c + 1) * CH])
                    qin_t[(c, b)] = qin

            def prefetch_mask(c):
                # on the sync HWDGE ring: queues FIFO behind the stage-A kin
                # loads, so the 2 MB mask transfers never compete with the
                # critical early input DMAs for HBM bandwidth
                if c < NCH and c not in mt_t:
                    mt = mpool.tile([P, NT, CH], dt.bfloat16)
                    nc.sync.dma_start(mt[:], mTt[:, :, c * CH:(c + 1) * CH])
                    mt_t[c] = mt

            prefetch_mask(0)
            prefetch_mask(1)
            prefetch_qin(0)
            prefetch_qin(1)

            pending = []

            def drain_one(alt):
                # deferred PSUM->SBUF drain of the previous unit's AV tile,
                # interleaved between this unit's exps so the ACT FIFO never
                # makes next-unit exps wait behind AV-gated copies
                if pending:
                    pso, pcol, pos = pending.pop(0)
                    ot = opool.tile([P, CH], dt.bfloat16)
                    nc.scalar.copy(ot[:], pso[:])
                    r0 = pos * P
                    eng = nc.sync if (pos % 2 == 0) else nc.gpsimd
                    eng.dma_start(out_d[r0:r0 + P, pcol:pcol + CH], ot[:])

            for i, (c, b) in enumerate(units):
                prefetch_qin(i + 2)
                if b == 0:
                    prefetch_mask(c + 1)
                mt = mt_t[c]
                col0 = b * S + c * CH
                qin = qin_t[(c, b)]
                last = (i >= len(units) - 2)

                E = epool.tile([P, NT, CH], dt.bfloat16, tag="E")
                for tt in range(NT):
                    ps = psA.tile([P, CH], dt.float32, tag="ps")
                    for et in range(NE):
                        nc.tensor.matmul(
                            ps[:], K2T[:, b, et, tt * P:(tt + 1) * P], qin[:, et, :],
                            start=(et == 0), stop=(et == NE - 1))
                    tmp = tpool.tile([P, CH], dt.float32)
                    nc.vector.scalar_tensor_tensor(
                        tmp[:], ps[:], float(SCALE), mt[:, tt, :],
                        op0=ALU.mult, op1=ALU.add)
                    nc.scalar.activation(
                        E[:, tt, :], tmp[:], AF.Exp,
                        bias=wb[:, b * NT + tt: b * NT + tt + 1], scale=1.0)
                    if tt in (1, 3, 5, 7):
                        drain_one(tt)

                # rowsum tree on gpsimd: keeps the DVE FIFO free for the
                # scalar_tensor_tensor drains that gate psA buffer reuse
                red = rpool.tile([P, NE, CH], dt.float32, tag="red")
                nc.gpsimd.tensor_add(red[:], E[:, 0:4, :], E[:, 4:8, :])
                nc.gpsimd.tensor_add(red[:], red[:], E[:, 8:12, :])
                nc.gpsimd.tensor_add(red[:], red[:], E[:, 12:16, :])
                nc.gpsimd.tensor_add(red[:, 0:2, :], red[:, 0:2, :], red[:, 2:4, :])
                accr = apool.tile([P, CH], dt.float32, tag="accr")
                nc.gpsimd.tensor_add(accr[:], red[:, 0, :], red[:, 1, :])
                rse = nc.sync if last else nc.gpsimd
                rse.dma_start(rs_d[:, col0:col0 + CH], accr[:])

                if i == len(units) - 1:
                    # last AV tile computed as two half-width chains into
                    # separate PSUM banks: the first half drains and ships
                    # while the second half still accumulates, shortening
                    # the post-matmul tail
                    for os_ in range(NE - 1):
                        pso = psO.tile([P, CH], dt.float32, tag="pso")
                        for tt in range(NT):
                            nc.tensor.matmul(
                                pso[:], V2[:, b, tt, os_ * P:(os_ + 1) * P],
                                E[:, tt, :],
                                start=(tt == 0), stop=(tt == NT - 1))
                        ot = opool.tile([P, CH], dt.bfloat16)
                        nc.scalar.copy(ot[:], pso[:])
                        r0 = os_ * P
                        eng = nc.sync if (os_ % 2 == 0) else nc.scalar
                        eng.dma_start(out_d[r0:r0 + P, col0:col0 + CH], ot[:])
                    r0 = (NE - 1) * P
                    for hh in range(2):
                        sl = slice(hh * (CH // 2), (hh + 1) * (CH // 2))
                        psh = psO.tile([P, CH // 2], dt.float32, tag="pso")
                        for tt in range(NT):
                            nc.tensor.matmul(
                                psh[:], V2[:, b, tt, (NE - 1) * P:NE * P],
                                E[:, tt, sl],
                                start=(tt == 0), stop=(tt == NT - 1))
                        oth = opool.tile([P, CH // 2], dt.bfloat16)
                        nc.scalar.copy(oth[:], psh[:])
                        eng = nc.sync if hh == 0 else nc.scalar
                        eng.dma_start(
                            out_d[r0:r0 + P, col0 + hh * (CH // 2):
                                  col0 + (hh + 1) * (CH // 2)], oth[:])
                    continue
                for os_ in range(NE):
                    pso = psO.tile([P, CH], dt.float32, tag="pso")
                    for tt in range(NT):
                        nc.tensor.matmul(
                            pso[:], V2[:, b, tt, os_ * P:(os_ + 1) * P],
                            E[:, tt, :],
                            start=(tt == 0), stop=(tt == NT - 1))
                    pending.append((pso, col0, os_))

    nc.compile()
    return nc


def kernel(q, k, v, mask, Wq, bq, Wk, bk, Wv, bv, Wo, bo):
    from concourse.bass_utils import run_bass_kernel_spmd
    import ml_dtypes

    bf16 = ml_dtypes.bfloat16

    q = np.asarray(q, np.float32)
    k = np.asarray(k, np.float32)
    v = np.asarray(v, np.float32)
    mask = np.asarray(mask)
    Wq = np.asarray(Wq, np.float32)
    Wk = np.asarray(Wk, np.float32)
    Wv = np.asarray(Wv, np.float32)
    Wo = np.asarray(Wo, np.float32)
    bq = np.asarray(bq, np.float32)
    bk = np.asarray(bk, np.float32)
    bv = np.asarray(bv, np.float32)
    bo = np.asarray(bo, np.float32)

    qT = np.ascontiguousarray(q.transpose(2, 0, 1).reshape(D, B * S)).astype(bf16)
    kT = np.ascontiguousarray(k.transpose(2, 0, 1).reshape(D, B * S)).astype(bf16)
    vT = np.ascontiguousarray(v.transpose(2, 0, 1).reshape(D, B * S)).astype(bf16)
    mT = np.where(mask.T == 1, np.float32(-1e9), np.float32(0.0)).astype(bf16)
    mT = np.ascontiguousarray(mT)

    kf = k.reshape(B * S, D)
    in_maps = []
    for h in range(H):
        Wq64 = Wq[h].astype(np.float64)
        Wk64 = Wk[h].astype(np.float64)
        Wv64 = Wv[h].astype(np.float64)
        Wo64 = Wo[h * D:(h + 1) * D, :].astype(np.float64)
        A = (Wk64 @ Wq64.T).astype(np.float32)       # lhsT for K2^T proj
        U = (Wv64 @ Wo64).astype(np.float32)         # rhs for V2 proj
        wvec = (kf @ (Wk[h] @ bq[h])) * SCALE        # per-key exp bias
        wb = np.ascontiguousarray(wvec.reshape(B * NT, P).T.astype(np.float32))
        in_maps.append({
            "qT": qT, "kT": kT, "vT": vT, "mT": mT,
            "ka": A.astype(bf16), "vu": U.astype(bf16), "wb": wb,
        })

    if "nc" not in _CACHE:
        _CACHE["nc"] = _build()
    nc = _CACHE["nc"]
    _CACHE["in_maps"] = in_maps

    res = run_bass_kernel_spmd(nc, in_maps, core_ids=list(range(H)))
    total = np.zeros((D, B * S), np.float64)
    for h in range(H):
        r = res.results[h]["rs"].sum(axis=0, dtype=np.float64)   # [B*S]
        total += res.results[h]["out"].astype(np.float64) / r[None, :]

    cvec = bo.astype(np.float64).copy()
    for h in range(H):
        cvec += bv[h].astype(np.float64) @ Wo[h * D:(h + 1) * D, :].astype(np.float64)
    total += cvec[:, None]
    return total.T.astype(np.float32).reshape(B, S, D)
